# revision 1
# baseline (speedup 1.0000x reference)
"""MatchLSTM attention kernel for 8 Trainium2 NeuronCores.

Reference computation (B=64, T=2048, D=512):
    G   = tanh(input_p@Wp.T + bp + input_q@Wq.T + bq + h_tm1@Wr.T + br)
    a   = softmax(G@w + match_b)            over T
    z   = sum_t a[:,t] * input_q[:,:,t]
    out = concat([input_p, z], -1)

Sharding: data-parallel over batch, 8 batches per core, weights replicated.

Per-core device pipeline (all matmul operands bf16, fp32 accumulation):
  - c^T[o,b] = (Wp.T;Wr.T;bias) matmuls against (input_p^T;h^T;ones)  [once]
  - X^T tiles [q,tok] via DMA-transpose; X natural tiles [tok,q] via DMA
  - G^T[o,tok] = Wq.T-chunk @ X^T-chunk (PE, fp32 PSUM)
  - tanh via ScalarE with per-partition bias c^T  -> bf16 SBUF
  - scores s[1,tok] = w-chunk.T @ tanhG (PE accum over o-chunks)
  - s transposed to columns via K=1 fp16 matmuls; exp(s+match_b) on ScalarE
    -> bf16, with sumexp accumulated for free via activation accum_out
  - z[1,512] = sum_j esc_j.T @ Xnat_j (PE, fp32 PSUM accumulation)
  - z scaled by 1/sumexp (VectorE), DMA out.  Softmax max-subtraction is
    skipped: |s| <= sum|w| + 1 < 25, exp stays well inside fp32 range.
"""

import sys

if "/opt/trn_rl_repo" not in sys.path:
    sys.path.insert(0, "/opt/trn_rl_repo")

import numpy as np
import ml_dtypes

N_CORES = 8
B, T, D = 64, 2048, 512
PB = B // N_CORES          # batches per core
KC = D // 128              # 4 contraction chunks of 128
NTT = T // 512             # 4 token tiles of 512
NJ = T // 128              # 16 token chunks of 128
CROWS = 2 * D + 128        # cw/cx rows: Wp.T, Wr.T, bias row + zero pad

BF16 = ml_dtypes.bfloat16

_CACHE: dict = {}


def _build_program():
    import concourse.bacc as bacc
    import concourse.tile as tile
    import concourse.mybir as mybir
    from concourse.bass import MemorySpace

    dt = mybir.dt
    F32 = dt.float32
    BF = dt.bfloat16
    AF = mybir.ActivationFunctionType

    nc = bacc.Bacc(
        "TRN2", target_bir_lowering=False, debug=False, num_devices=N_CORES
    )

    xq_d = nc.dram_tensor("xq", [PB, T, D], BF, kind="ExternalInput")
    wq_d = nc.dram_tensor("wqt", [D, D], BF, kind="ExternalInput")      # Wq.T [q,o]
    cw_d = nc.dram_tensor("cw", [CROWS, D], BF, kind="ExternalInput")   # [Wp.T;Wr.T;bias;0]
    cx_d = nc.dram_tensor("cx", [CROWS, PB], BF, kind="ExternalInput")  # [ip.T;h.T;1;0]
    wcol_d = nc.dram_tensor("wcol", [D, 1], BF, kind="ExternalInput")
    mb_d = nc.dram_tensor("mb", [128, 1], F32, kind="ExternalInput")    # match_b bcast
    z_d = nc.dram_tensor("z", [1, PB * D], F32, kind="ExternalOutput")

    NKC = CROWS // 128  # 9 contraction chunks for the c matmuls

    F16 = dt.float16

    with tile.TileContext(nc) as tc:
        with (
            tc.tile_pool(name="consts", bufs=1) as consts,
            tc.tile_pool(name="xT_p", bufs=3) as xT_pool,
            tc.tile_pool(name="xnat_p", bufs=3) as xnat_pool,
            tc.tile_pool(name="tanh_p", bufs=8) as tanh_pool,
            tc.tile_pool(name="srow_p", bufs=3) as srow_pool,
            tc.tile_pool(name="esc_p", bufs=3) as esc_pool,
            tc.tile_pool(name="small_p", bufs=2) as small_pool,
            tc.tile_pool(name="zout_p", bufs=1) as zout_pool,
            tc.tile_pool(name="pG", bufs=2, space=MemorySpace.PSUM) as pG,
            tc.tile_pool(name="pS", bufs=2, space=MemorySpace.PSUM) as pS,
            tc.tile_pool(name="pZ", bufs=1, space=MemorySpace.PSUM) as pZ,
            tc.tile_pool(name="pM", bufs=1, space=MemorySpace.PSUM) as pM,
        ):
            # ---- constants (DMA order = criticality order) -----------------
            cw_s = consts.tile([128, NKC, D], BF, tag="cw", name="cw_s")
            nc.sync.dma_start(out=cw_s, in_=cw_d.rearrange("(c p) o -> p c o", p=128))
            cx_s = consts.tile([128, NKC, PB], BF, tag="cx", name="cx_s")
            nc.sync.dma_start(out=cx_s, in_=cx_d.rearrange("(c p) b -> p c b", p=128))
            wq_s = consts.tile([128, KC, D], BF, tag="wq", name="wq_s")
            nc.sync.dma_start(out=wq_s, in_=wq_d.rearrange("(c p) o -> p c o", p=128))
            wcol_s = consts.tile([128, KC, 1], BF, tag="wcol", name="wcol_s")
            nc.sync.dma_start(out=wcol_s, in_=wcol_d.rearrange("(c p) o -> p c o", p=128))
            mb_s = consts.tile([128, 1], F32, tag="mb", name="mb_s")
            nc.sync.dma_start(out=mb_s, in_=mb_d[:, :])
            ones128 = consts.tile([128, 1], F32, tag="ones128", name="ones128")
            nc.vector.memset(ones128, 1.0)
            ones_f16 = consts.tile([1, 1], F16, tag="ones_f16", name="ones_f16")
            nc.vector.memset(ones_f16, 1.0)
            # warm the ACT table set (tanh/exp share one set) off the critical path
            dummy_s = consts.tile([1, 1], F32, tag="dummy", name="dummy_s")
            nc.scalar.activation(
                out=dummy_s, in_=ones_f16, func=AF.Tanh, bias=0.0, scale=1.0
            )

            # ---- c^T[o, b] for all batches (once) --------------------------
            c_ps = pM.tile([128, KC, PB], F32, tag="misc", name="c_ps")
            for oc in range(KC):
                for k in range(NKC):
                    nc.tensor.matmul(
                        c_ps[:, oc, :],
                        cw_s[:, k, oc * 128 : (oc + 1) * 128],
                        cx_s[:, k, :],
                        start=(k == 0),
                        stop=(k == NKC - 1),
                    )
            cT_s = consts.tile([128, KC, PB], F32, tag="cT", name="cT_s")
            nc.vector.tensor_copy(out=cT_s, in_=c_ps)

            zout_s = zout_pool.tile([1, PB, D], F32, tag="zout", name="zout_s")

            # ---- per-batch pipeline ---------------------------------------
            for b in range(PB):
                xT = xT_pool.tile([128, KC, T], BF, tag="xT", name="xT")
                # batch 0 is latency-critical: land the first half-T of each
                # q-chunk sooner by splitting the transposes.
                nh = 2 if b == 0 else 1
                for h in range(nh):
                    for qc in range(KC):
                        nc.sync.dma_start(
                            out=xT[:, qc, h * (T // nh) : (h + 1) * (T // nh)],
                            in_=xq_d[
                                b,
                                h * (T // nh) : (h + 1) * (T // nh),
                                qc * 128 : (qc + 1) * 128,
                            ],
                            transpose=True,
                        )
                xnat = xnat_pool.tile([128, NJ, D], BF, tag="xnat", name="xnat")
                nc.sync.dma_start(
                    out=xnat, in_=xq_d[b].rearrange("(i p) q -> p i q", p=128)
                )

                s_cat = srow_pool.tile([1, T], F16, tag="scat", name="s_cat")
                esc = esc_pool.tile([128, NJ], BF, tag="esc", name="esc")
                pesum = small_pool.tile([128, 2], F32, tag="pesum", name="pesum")
                z_ps = pZ.tile([1, D], F32, tag="z", name="z_ps")
                # token tiles processed in pairs sharing one [128,1024] PSUM
                # G tile (2 banks): same Wq chunk stays loaded across the pair
                # and tanh runs once per 1024 tokens.
                for tp in range(NTT // 2):
                    sT_ps = pM.tile([128, NJ // 2], F32, tag="misc", name="sT_ps")
                    sc_pair = [
                        pS.tile([1, 512], F32, tag="s", name="sc_ps")
                        for _ in range(2)
                    ]
                    for oc in range(KC):
                        g_ps = pG.tile([128, 1024], F32, tag="g", name="g_ps")
                        for qc in range(KC):
                            for i in range(2):
                                tt = tp * 2 + i
                                nc.tensor.matmul(
                                    g_ps[:, i * 512 : (i + 1) * 512],
                                    wq_s[:, qc, oc * 128 : (oc + 1) * 128],
                                    xT[:, qc, tt * 512 : (tt + 1) * 512],
                                    start=(qc == 0),
                                    stop=(qc == KC - 1),
                                )
                        th = tanh_pool.tile([128, 1024], BF, tag="th", name="th")
                        nc.scalar.activation(
                            out=th,
                            in_=g_ps,
                            func=AF.Tanh,
                            bias=cT_s[:, oc, b : b + 1],
                            scale=1.0,
                        )
                        for i in range(2):
                            nc.tensor.matmul(
                                sc_pair[i],
                                wcol_s[:, oc, :],
                                th[:, i * 512 : (i + 1) * 512],
                                start=(oc == 0),
                                stop=(oc == KC - 1),
                            )
                    for i in range(2):
                        tt = tp * 2 + i
                        nc.vector.tensor_copy(
                            out=s_cat[:, tt * 512 : (tt + 1) * 512], in_=sc_pair[i]
                        )
                        # transpose scores into columns (K=1 fp16 matmuls)
                        for jj in range(4):
                            j = tt * 4 + jj
                            nc.tensor.matmul(
                                sT_ps[:, j - tp * 8 : j - tp * 8 + 1],
                                s_cat[:, j * 128 : (j + 1) * 128],
                                ones_f16,
                                start=True,
                                stop=True,
                            )
                    # exp + its half of the z accumulation start mid-batch
                    nc.scalar.activation(
                        out=esc[:, tp * 8 : (tp + 1) * 8],
                        in_=sT_ps,
                        func=AF.Exp,
                        bias=mb_s,
                        scale=1.0,
                        accum_out=pesum[:, tp : tp + 1],
                    )
                    for j in range(tp * 8, (tp + 1) * 8):
                        nc.tensor.matmul(
                            z_ps,
                            esc[:, j : j + 1],
                            xnat[:, j, :],
                            start=(j == 0),
                            stop=(j == NJ - 1),
                        )

                se_ps = pM.tile([1, 2], F32, tag="misc", name="se_ps")
                nc.tensor.matmul(se_ps, ones128, pesum, start=True, stop=True)
                se_sb = small_pool.tile([1, 2], F32, tag="sesb", name="se_sb")
                nc.vector.tensor_copy(out=se_sb, in_=se_ps)
                se_tot = small_pool.tile([1, 1], F32, tag="setot", name="se_tot")
                nc.vector.tensor_add(se_tot, se_sb[:, 0:1], se_sb[:, 1:2])
                rse_s = small_pool.tile([1, 1], F32, tag="rse", name="rse_s")
                nc.vector.reciprocal(out=rse_s, in_=se_tot)

                nc.vector.tensor_scalar_mul(
                    out=zout_s[:, b, :], in0=z_ps, scalar1=rse_s
                )

            nc.sync.dma_start(out=z_d[:, :], in_=zout_s)

    nc.compile()
    return nc


def _get_program():
    if "nc" not in _CACHE:
        _CACHE["nc"] = _build_program()
    return _CACHE["nc"]


def kernel(**inputs) -> np.ndarray:
    from concourse import bass_utils

    inp = {k: np.asarray(v) for k, v in inputs.items()}
    input_p = inp["input_p"].astype(np.float32)
    input_q = inp["input_q"].astype(np.float32)
    h_tm1 = inp["h_tm1"].astype(np.float32)
    Wp, Wq, Wr = inp["Wp"], inp["Wq"], inp["Wr"]
    bp, bq, br = inp["bp"], inp["bq"], inp["br"]
    w = inp["w"]
    mb = float(np.asarray(inp["match_b"]).reshape(-1)[0])

    # shared (weight) tensors
    wqt = np.ascontiguousarray(Wq.T).astype(BF16)
    cw = np.zeros((CROWS, D), dtype=BF16)
    cw[:D] = Wp.T.astype(BF16)
    cw[D : 2 * D] = Wr.T.astype(BF16)
    cw[2 * D] = (bp.astype(np.float32) + bq + br).astype(BF16)
    wcol = np.ascontiguousarray(w.reshape(D, 1)).astype(BF16)
    mb_arr = np.full((128, 1), mb, dtype=np.float32)

    nc = _get_program()

    in_maps = []
    for c in range(N_CORES):
        s = slice(c * PB, (c + 1) * PB)
        cx = np.zeros((CROWS, PB), dtype=BF16)
        cx[:D] = input_p[s].T.astype(BF16)
        cx[D : 2 * D] = h_tm1[s].T.astype(BF16)
        cx[2 * D] = 1.0
        in_maps.append(
            {
                "xq": np.ascontiguousarray(input_q[s]).astype(BF16),
                "wqt": wqt,
                "cw": cw,
                "cx": cx,
                "wcol": wcol,
                "mb": mb_arr,
            }
        )

    res = bass_utils.run_bass_kernel_spmd(
        nc, in_maps, core_ids=list(range(N_CORES))
    )
    z = np.concatenate(
        [
            np.asarray(res.results[c]["z"], dtype=np.float32).reshape(PB, D)
            for c in range(N_CORES)
        ],
        axis=0,
    )
    return np.concatenate([input_p, z], axis=1)



# revision 11
# speedup vs baseline: 2.3042x; 2.3042x over previous
"""MatchLSTM attention kernel for 8 Trainium2 NeuronCores.

Reference computation (B=64, T=2048, D=512):
    G   = tanh(input_p@Wp.T + bp + input_q@Wq.T + bq + h_tm1@Wr.T + br)
    a   = softmax(G@w + match_b)            over T
    z   = sum_t a[:,t] * input_q[:,:,t]
    out = concat([input_p, z], -1)

Sharding: data-parallel over batch, 8 batches per core, weights replicated.

Per-core pipeline (fp8 e4m3 matmul operands, DoubleRow perf mode = 2 k-tiles
of 128 per instruction at 0.5 cycles/col, fp32 PSUM accumulation):
  - c^T[o,b] = (Wp.T;Wr.T;bias) @ (ip.T;h.T;1)  [once, fp8 DoubleRow]
  - X^T tiles [q,tok] and X natural tiles [tok,q] DMA'd fp8 (host provides
    both layouts pre-transposed; no DMA-transpose needed)
  - G^T[o,tok] = Wq.T @ X^T (fp8 DoubleRow), tanh on ScalarE with
    per-partition bias c^T -> fp8 SBUF
  - scores transposed for free: sT[tok,1] = th-chunk (stationary) @ wcol
    (moving, N=1); w is scaled by 32 host-side to stay normal in fp8
  - exp(sT/32 - 1.5) on ScalarE -> fp8 esc columns (match_b cancels in
    softmax; the -1.5 shift keeps exp well within fp8 range and cancels too)
  - z^T[q,1] += xnat-chunk (stationary) @ esc pair (moving, N=1), and
    sumexp  += esc pair (stationary) @ ones (consistent with quantized esc)
  - z^T and sumexp DMA'd out unnormalized; host divides and re-lays out.
Scores/z/sumexp matmuls move N=1 columns so their PE cost is ~nil; PE time
is the G matmul; ScalarE tanh is the bottleneck. PE consumers of ScalarE
outputs are emitted a few G-stages late so the in-order PE queue never
parks on a ScalarE dependency.
"""

import sys

if "/opt/trn_rl_repo" not in sys.path:
    sys.path.insert(0, "/opt/trn_rl_repo")

import numpy as np
import ml_dtypes

N_CORES = 8
B, T, D = 64, 2048, 512
PB = B // N_CORES          # batches per core
KC = D // 128              # 4 contraction chunks of 128
NJ = T // 128              # 16 token chunks of 128
CROWS = 1280               # cw/cx rows: Wp.T, Wr.T, bias row, zero pad (10*128)
NKP = CROWS // 256         # 5 DoubleRow pairs for the c matmul

F8 = ml_dtypes.float8_e4m3
WSCALE = 32.0
ESHIFT = -1.5

_CACHE: dict = {}


def _build_program():
    import concourse.bacc as bacc
    import concourse.tile as tile
    import concourse.mybir as mybir
    from concourse.bass import MemorySpace

    dt = mybir.dt
    F32 = dt.float32
    FP8 = dt.float8e4
    AF = mybir.ActivationFunctionType
    DR = mybir.MatmulPerfMode.DoubleRow

    nc = bacc.Bacc(
        "TRN2", target_bir_lowering=False, debug=False, num_devices=N_CORES
    )

    xqt_d = nc.dram_tensor("xqt", [PB, D, T], FP8, kind="ExternalInput")
    xq_d = nc.dram_tensor("xq", [PB, T, D], FP8, kind="ExternalInput")
    wq_d = nc.dram_tensor("wqt", [D, D], FP8, kind="ExternalInput")     # Wq.T [q,o]
    cw_d = nc.dram_tensor("cw", [CROWS, D], FP8, kind="ExternalInput")  # [Wp.T;Wr.T;bias;0]
    cx_d = nc.dram_tensor("cx", [CROWS, PB], FP8, kind="ExternalInput") # [ip.T;h.T;1;0]
    wcol_d = nc.dram_tensor("wcol", [D, 1], FP8, kind="ExternalInput")  # 32*w
    z_d = nc.dram_tensor("z", [128, PB * KC], F32, kind="ExternalOutput")
    se_d = nc.dram_tensor("se", [1, PB], F32, kind="ExternalOutput")

    with tile.TileContext(nc) as tc:
        with (
            tc.tile_pool(name="consts", bufs=1) as consts,
            tc.tile_pool(name="xT_p", bufs=3) as xT_pool,
            tc.tile_pool(name="xnat_p", bufs=3) as xnat_pool,
            tc.tile_pool(name="tanh_p", bufs=3) as tanh_pool,
            tc.tile_pool(name="esc_p", bufs=2) as esc_pool,
            tc.tile_pool(name="out_p", bufs=1) as out_pool,
            # PSUM budget (8 banks of 2KB/partition):
            #   pG 2 bufs x [128,1024]f32 (2 banks each)           = 4 banks
            #   pS 2 bufs x 1 bank (scores sT; also c_ps at setup) = 2 banks
            #   pZ 2 bufs x 1 bank (z^T cols + sumexp col)         = 2 banks
            tc.tile_pool(name="pG", bufs=2, space=MemorySpace.PSUM) as pG,
            tc.tile_pool(name="pS", bufs=2, space=MemorySpace.PSUM) as pS,
            tc.tile_pool(name="pZ", bufs=2, space=MemorySpace.PSUM) as pZ,
        ):
            # ---- constants (DMA order = criticality order) -----------------
            wq_s = consts.tile([128, KC, D], FP8, tag="wq", name="wq_s")
            nc.sync.dma_start(out=wq_s, in_=wq_d.rearrange("(c p) o -> p c o", p=128))

            xT0 = xT_pool.tile([128, KC, T], FP8, tag="xT", name="xT")
            nc.sync.dma_start(
                out=xT0[:, :, 0:1024],
                in_=xqt_d[0, :, 0:1024].rearrange("(c p) t -> p c t", p=128),
            )

            cw_s = consts.tile([128, NKP, 2, D], FP8, tag="cw", name="cw_s")
            nc.sync.dma_start(
                out=cw_s, in_=cw_d.rearrange("(c two p) o -> p c two o", p=128, two=2)
            )
            cx_s = consts.tile([128, NKP, 2, PB], FP8, tag="cx", name="cx_s")
            nc.sync.dma_start(
                out=cx_s, in_=cx_d.rearrange("(c two p) b -> p c two b", p=128, two=2)
            )
            wcol_s = consts.tile([128, KC, 1], FP8, tag="wcol", name="wcol_s")
            nc.sync.dma_start(out=wcol_s, in_=wcol_d.rearrange("(c p) o -> p c o", p=128))

            nc.sync.dma_start(
                out=xT0[:, :, 1024:2048],
                in_=xqt_d[0, :, 1024:2048].rearrange("(c p) t -> p c t", p=128),
            )
            xnat0 = xnat_pool.tile([128, NJ, D], FP8, tag="xnat", name="xnat")
            nc.sync.dma_start(
                out=xnat0, in_=xq_d[0].rearrange("(j p) q -> p j q", p=128)
            )

            ones8 = consts.tile([128, 2, 1], FP8, tag="ones8", name="ones8")
            nc.vector.memset(ones8, 1.0)
            eb_s = consts.tile([128, 1], F32, tag="eb", name="eb_s")
            nc.vector.memset(eb_s, ESHIFT)

            # ---- c^T[o, b] for all batches (once) --------------------------
            # PSUM `start` marks the whole 2KB bank pending-zero, so exactly
            # one start per bank: the first instruction touching it.
            c_ps = pS.tile([128, KC, PB], F32, tag="sT", name="c_ps")
            for oc in range(KC):
                for k in range(NKP):
                    nc.tensor.matmul(
                        c_ps[:, oc, :],
                        cw_s[:, k, :, oc * 128 : (oc + 1) * 128],
                        cx_s[:, k, :, :],
                        start=(oc == 0 and k == 0),
                        stop=(oc == KC - 1 and k == NKP - 1),
                        perf_mode=DR,
                        skip_group_check=True,
                    )
            cT_s = consts.tile([128, KC, PB], F32, tag="cT", name="cT_s")
            nc.vector.tensor_copy(out=cT_s, in_=c_ps)

            zts = out_pool.tile([128, PB, KC], F32, tag="zts", name="zts")
            se_all = out_pool.tile([1, PB], F32, tag="seall", name="se_all")

            # ---- per-batch software-pipelined loop ------------------------
            pending: list = []  # (due_global_stage, seq, fn)
            gidx = 0
            seq = 0

            state: dict = {}

            def flush(upto):
                nonlocal pending
                ready = sorted((p for p in pending if p[0] <= upto))
                pending = [p for p in pending if p[0] > upto]
                for _, _, fn in ready:
                    fn()

            def defer(due, fn):
                nonlocal seq
                pending.append((due, seq, fn))
                seq += 1

            def emit_scores(b, tp):
                st = state[b]
                th = st["th"][tp]
                sT_ps = pS.tile([128, 8], F32, tag="sT", name="sT_ps")
                for tc_ in range(8):
                    for g in range(2):
                        nc.tensor.matmul(
                            sT_ps[:, tc_ : tc_ + 1],
                            th[:, 2 * g : 2 * g + 2, tc_ * 128 : (tc_ + 1) * 128],
                            wcol_s[:, 2 * g : 2 * g + 2, :],
                            start=(tc_ == 0 and g == 0),
                            stop=(tc_ == 7 and g == 1),
                            perf_mode=DR,
                            skip_group_check=True,
                        )
                esc = st["esc"]
                nc.scalar.activation(
                    out=esc[:, tp * 8 : (tp + 1) * 8, :],
                    in_=sT_ps,
                    func=AF.Exp,
                    bias=eb_s,
                    scale=1.0 / WSCALE,
                )

            def emit_z(b, tp):
                st = state[b]
                esc, xnat, zse = st["esc"], st["xnat"], st["zse"]
                for jp in range(tp * 4, tp * 4 + 4):
                    for qc in range(KC):
                        nc.tensor.matmul(
                            zse[:, qc : qc + 1],
                            xnat[:, 2 * jp : 2 * jp + 2, qc * 128 : (qc + 1) * 128],
                            esc[:, 2 * jp : 2 * jp + 2, :],
                            start=(jp == 0 and qc == 0),
                            stop=False,
                            perf_mode=DR,
                            skip_group_check=True,
                        )
                    for j in (2 * jp, 2 * jp + 1):
                        nc.tensor.matmul(
                            zse[0:1, KC : KC + 1],
                            esc[:, j, :],
                            ones8[:, 0, :],
                            start=False,
                            stop=(j == NJ - 1),
                            skip_group_check=True,
                        )

            def emit_out(b):
                st = state[b]
                nc.vector.tensor_copy(out=zts[:, b, :], in_=st["zse"][:, 0:KC])
                nc.vector.tensor_copy(
                    out=se_all[:, b : b + 1], in_=st["zse"][0:1, KC : KC + 1]
                )

            for b in range(PB):
                if b == 0:
                    xT, xnat = xT0, xnat0
                else:
                    xT = xT_pool.tile([128, KC, T], FP8, tag="xT", name="xT")
                    nc.sync.dma_start(
                        out=xT, in_=xqt_d[b].rearrange("(c p) t -> p c t", p=128)
                    )
                    xnat = xnat_pool.tile([128, NJ, D], FP8, tag="xnat", name="xnat")
                    nc.sync.dma_start(
                        out=xnat, in_=xq_d[b].rearrange("(j p) q -> p j q", p=128)
                    )
                state[b] = {
                    "xnat": xnat,
                    "th": {},
                    "esc": esc_pool.tile([128, NJ, 1], FP8, tag="esc", name="esc"),
                    "zse": pZ.tile([128, KC + 1], F32, tag="zse", name="zse_ps"),
                }

                for tp in range(2):
                    th = tanh_pool.tile([128, KC, 1024], FP8, tag="th", name="th")
                    state[b]["th"][tp] = th
                    for oc in range(KC):
                        g_ps = pG.tile([128, 1024], F32, tag="g", name="g_ps")
                        for h in range(2):          # one 2KB bank per h
                            for kg in range(2):
                                for i in range(2):
                                    nc.tensor.matmul(
                                        g_ps[:, h * 512 + i * 256 : h * 512 + (i + 1) * 256],
                                        wq_s[:, 2 * kg : 2 * kg + 2, oc * 128 : (oc + 1) * 128],
                                        xT[:, 2 * kg : 2 * kg + 2,
                                           tp * 1024 + h * 512 + i * 256 :
                                           tp * 1024 + h * 512 + (i + 1) * 256],
                                        start=(kg == 0 and i == 0),
                                        stop=(kg == 1 and i == 1),
                                        perf_mode=DR,
                                        skip_group_check=True,
                                    )
                        nc.scalar.activation(
                            out=th[:, oc, :],
                            in_=g_ps,
                            func=AF.Tanh,
                            bias=cT_s[:, oc, b : b + 1],
                            scale=1.0,
                        )
                        gidx += 1
                        flush(gidx)

                    # schedule this tp's consumers into the future stream
                    bb, tt = b, tp
                    defer(gidx + 2, lambda bb=bb, tt=tt: emit_scores(bb, tt))
                    defer(gidx + 4, lambda bb=bb, tt=tt: emit_z(bb, tt))
                    if tp == 1:
                        defer(gidx + 5, lambda bb=bb: emit_out(bb))

            # drain the pipeline tail
            flush(10**9)

            nc.sync.dma_start(out=z_d[:, :], in_=zts.rearrange("p b c -> p (b c)"))
            nc.sync.dma_start(out=se_d[:, :], in_=se_all)

    nc.compile()
    return nc


def _get_program():
    if "nc" not in _CACHE:
        _CACHE["nc"] = _build_program()
    return _CACHE["nc"]


def kernel(**inputs) -> np.ndarray:
    from concourse import bass_utils

    inp = {k: np.asarray(v) for k, v in inputs.items()}
    input_p = inp["input_p"].astype(np.float32)
    input_q = inp["input_q"].astype(np.float32)
    h_tm1 = inp["h_tm1"].astype(np.float32)
    Wp, Wq, Wr = inp["Wp"], inp["Wq"], inp["Wr"]
    bp, bq, br = inp["bp"], inp["bq"], inp["br"]
    w = inp["w"]

    # shared (weight) tensors
    wqt = np.ascontiguousarray(np.asarray(Wq, np.float32).T).astype(F8)
    cw = np.zeros((CROWS, D), dtype=F8)
    cw[:D] = np.asarray(Wp, np.float32).T.astype(F8)
    cw[D : 2 * D] = np.asarray(Wr, np.float32).T.astype(F8)
    cw[2 * D] = (
        np.asarray(bp, np.float32) + np.asarray(bq, np.float32) + np.asarray(br, np.float32)
    ).astype(F8)
    wcol = np.ascontiguousarray(
        (WSCALE * np.asarray(w, np.float32)).reshape(D, 1)
    ).astype(F8)

    nc = _get_program()

    in_maps = []
    for c in range(N_CORES):
        s = slice(c * PB, (c + 1) * PB)
        cx = np.zeros((CROWS, PB), dtype=F8)
        cx[:D] = input_p[s].T.astype(F8)
        cx[D : 2 * D] = h_tm1[s].T.astype(F8)
        cx[2 * D] = 1.0
        in_maps.append(
            {
                "xqt": np.ascontiguousarray(input_q[s].transpose(0, 2, 1)).astype(F8),
                "xq": np.ascontiguousarray(input_q[s]).astype(F8),
                "wqt": wqt,
                "cw": cw,
                "cx": cx,
                "wcol": wcol,
            }
        )

    res = bass_utils.run_bass_kernel_spmd(
        nc, in_maps, core_ids=list(range(N_CORES))
    )
    zs = []
    for c in range(N_CORES):
        zt = np.asarray(res.results[c]["z"], dtype=np.float32).reshape(128, PB, KC)
        se = np.asarray(res.results[c]["se"], dtype=np.float32).reshape(PB)
        # z[b, qc*128 + p] = zt[p, b, qc] / se[b]
        zs.append(zt.transpose(1, 2, 0).reshape(PB, D) / se[:, None])
    z = np.concatenate(zs, axis=0)
    return np.concatenate([input_p, z], axis=1)
